# revision 17
# baseline (speedup 1.0000x reference)
"""Causal self-attention (B=2, T=2048, C=1024, H=16) on 8 TRN2 NeuronCores.

Sharding: core c -> batch b = c // 4, head group = heads [4*(c%4) .. 4*(c%4)+4).
Each core computes qkv for its 4 heads on its batch, causal attention, and a
row-parallel partial of the output projection (over its 256 head channels).
The host sums the 4 partials per batch; b_proj/4 is added on-device so the sum
reproduces a single b_proj add.  Output partials stream back in bf16.

All device tensors are pre-transposed on the host so the kernel never
transposes on-chip:
  xt   [C, T]    = x[b].T                     (bf16)
  wqkt [C, 512]  = w_attn[qk rows].T          (bf16)  cols: q_h0 q_h1 q_h2 q_h3 k_h0..k_h3
  wvt  [C, 256]  = w_attn[v rows].T           (bf16)
  wpt  [256, C]  = w_proj[:, head cols].T     (bf16)
  bias [128,268] = packed bqk(4) | bp(8) | bv(256) columns (fp32)
  out_t[C, T]    = partial (x @ w_proj.T).T   (bf16)

Round 3 (vs the round-2 baseline at ~194us):
  * input DMAs coalesced into 7 big multi-dim-AP transfers (xt in four
    1MB t-blocks on sync, wqk/wv on scalar, bias/wp on gpsimd) so the
    first qk group can start ~6us in instead of ~22us;
  * softmax 1/den moved off ScalarE: DVE copies the PSUM den row to bf16,
    a K=1 matmul broadcasts den across 64 partitions, and
    reciprocal_approx_fast (custom DVE op, ~18 bits) inverts it -- the
    Ln/Exp activation chain is gone (ScalarE now runs score exps only);
  * av pair kept in one 2-bank PSUM tile [128,1024] (den row spans both
    heads, one DVE copy per unit);
  * proj outputs accumulate per tq-group into resident [128,4096] bf16
    tiles, written back with one DMA per group (g=3 split in halves), so
    the tail is a short dense burst instead of 8 DMA-gated proj groups.

On-chip dataflow (per head pair, row/col layouts chosen so the TensorE
contraction dim is always the partition dim and no transposes are needed):
  qT,kT [d, t] -> S^T[tk, tq] (two heads packed in the 128-wide array via
  row tiling) -> exp on ScalarE (scale=1/8 folded in) -> causal mask via
  static 0/1 mask multiply on DVE -> AV matmul with V augmented by a ones
  column (denominator accumulates in row 64 of PSUM for free) -> bcast +
  reciprocal -> normalize -> projection (stays transposed).
"""

import os
import sys
import types

import numpy as np
import ml_dtypes

import concourse.bass as bass
import concourse.mybir as mybir
import concourse.tile as tile
from concourse import bacc
from concourse.hw_specs import get_activation_tables

BF16 = ml_dtypes.bfloat16


class _Bacc(bacc.Bacc):
    """Bacc that steers Exp/Ln activations to the combined
    natural_log_exp_and_others table set so the kernel never swaps
    activation tables (set ids keep their act_info.json positions)."""

    def insert_act_table_loads(self):
        import bass_rust as _br
        import concourse.mybir as _mybir

        has_activation = any(
            isinstance(i, _mybir.InstActivation)
            for b in self.main_func.blocks
            for i in b.instructions
        )
        if not has_activation:
            return
        combined = {"natural_log_exp_and_others"}
        steer = {_mybir.ActivationFunctionType.Exp, _mybir.ActivationFunctionType.Ln}
        tables = []
        for name, fns in get_activation_tables(self.m.arch).items():
            if name not in combined:
                fns = {f for f in fns if f not in steer}
            tables.append((name, set(fns)))
        _br.insert_act_table_loads(self, tables)

B, T, C = 2, 2048, 1024
H = 16
DH = 64
N_CORES = 8
HEADS_PER_CORE = 4
TQ = 512          # tq tile (moving dim of scores/AV matmuls)
TK = 128          # tk tile (PSUM partition dim of S^T)
NG = T // TQ      # 4 tq tiles
NKT = T // TK     # 16 tk tiles
NC_ = C // 128    # 8 contraction tiles for the qkv matmuls
FP32 = mybir.dt.float32
BF16_DT = mybir.dt.bfloat16
import os as _os
DEPTH = int(_os.environ.get("K_DEPTH", "4"))
POPS_EVERY = int(_os.environ.get("K_POPS_EVERY", "2"))
BOUNDARY_POPS = int(_os.environ.get("K_BPOPS", "2"))
WARMUP_MMS = int(_os.environ.get("K_WARMUP", "40"))
PT_BUFS = int(_os.environ.get("K_PT_BUFS", "8"))


def _ensure_axon_hooks_stub():
    """bass_utils imports antenv.axon_hooks when trace is requested (even via
    the BASS_TRACE env var). The container's antenv stub lacks that module, so
    install a minimal one to keep the no-trace fallback path working."""
    try:
        import antenv  # noqa: F401
    except ImportError:
        return
    if "antenv.axon_hooks" in sys.modules:
        return
    try:
        import antenv.axon_hooks  # noqa: F401
        return
    except ImportError:
        pass
    mod = types.ModuleType("antenv.axon_hooks")
    mod._hook = None

    def set_axon_ntff_profile_hook(h):
        mod._hook = h

    def get_axon_ntff_profile_hook():
        return mod._hook

    mod.set_axon_ntff_profile_hook = set_axon_ntff_profile_hook
    mod.get_axon_ntff_profile_hook = get_axon_ntff_profile_hook
    sys.modules["antenv.axon_hooks"] = mod
    import antenv as _a

    _a.axon_hooks = mod


def build_bass():
    """Emit the single-core SPMD Bass module (same program on all 8 cores)."""
    from collections import deque
    from contextlib import ExitStack

    nc = _Bacc("TRN2", target_bir_lowering=False, debug=False)

    xt = nc.declare_dram_parameter("xt", [C, T], BF16_DT, isOutput=False).ap()
    wqkt = nc.declare_dram_parameter("wqkt", [C, 512], BF16_DT, isOutput=False).ap()
    wvt = nc.declare_dram_parameter("wvt", [C, 256], BF16_DT, isOutput=False).ap()
    wpt = nc.declare_dram_parameter("wpt", [256, C], BF16_DT, isOutput=False).ap()
    bias = nc.declare_dram_parameter("bias", [128, 268], FP32, isOutput=False).ap()
    out_t = nc.declare_dram_parameter("out_t", [C, T], BF16_DT, isOutput=True).ap()

    Exp = mybir.ActivationFunctionType.Exp
    mult = mybir.AluOpType.mult
    add = mybir.AluOpType.add
    is_ge = mybir.AluOpType.is_ge

    with tile.TileContext(nc) as tc, ExitStack() as ctx:
        res = ctx.enter_context(tc.tile_pool(name="resident", bufs=1))

        # --- resident tiles (one big tile per input, sliced into views) ----
        xt_big = res.tile([128, NC_ * T], BF16_DT, tag="xt", name="xt_big")
        xt_t = [xt_big[:, T * i : T * (i + 1)] for i in range(NC_)]
        wqk_big = res.tile([128, NC_ * 512], BF16_DT, tag="wqk", name="wqk_big")
        wqk_t = [wqk_big[:, 512 * i : 512 * (i + 1)] for i in range(NC_)]
        wv_big = res.tile([128, NC_ * 256], BF16_DT, tag="wv", name="wv_big")
        wv_t = [wv_big[:, 256 * i : 256 * (i + 1)] for i in range(NC_)]
        wp_big = res.tile([128, 2 * C], BF16_DT, tag="wp", name="wp_big")
        wp_t = [wp_big[:, C * i : C * (i + 1)] for i in range(2)]
        bias_t = res.tile([128, 268], FP32, tag="bias", name="bias_t")
        bqk_t = [bias_t[:, j : j + 1] for j in range(4)]
        bp_t = [bias_t[:, 4 + j : 5 + j] for j in range(8)]
        bv_t = bias_t[:, 12:268]
        osb_t = [res.tile([128, NC_ * TQ], BF16_DT, tag=f"osb{g}", name=f"osb{g}")
                 for g in range(NG)]

        # --- PE warm-up first: dense zero matmuls while the DMAs stream in,
        # so the HAM clock gate opens before real compute starts ------------
        warm_sb = res.tile([128, 512], BF16_DT, tag="warm", name="warm_sb")
        nc.vector.memset(warm_sb[:], 0.0)

        sc_ps = ctx.enter_context(tc.tile_pool(name="sc_ps", bufs=2, space="PSUM"))
        av_ps = ctx.enter_context(tc.tile_pool(name="av_ps", bufs=1, space="PSUM"))
        qv_ps = ctx.enter_context(tc.tile_pool(name="qv_ps", bufs=1, space="PSUM"))
        bp_ps = ctx.enter_context(tc.tile_pool(name="bp_ps", bufs=1, space="PSUM"))
        pt_pool = ctx.enter_context(tc.tile_pool(name="pt_pool", bufs=PT_BUFS))
        riv_pool = ctx.enter_context(tc.tile_pool(name="riv", bufs=2))
        bcs_pool = ctx.enter_context(tc.tile_pool(name="bcs", bufs=2))
        scr_pool = ctx.enter_context(tc.tile_pool(name="scr", bufs=2))

        warm_ps = qv_ps.tile([128, 512], FP32, tag="qv", name="warm_ps")
        for i in range(WARMUP_MMS):
            nc.tensor.matmul(
                warm_ps[:], lhsT=warm_sb[:, 0:128], rhs=warm_sb[:],
                start=(i == 0), stop=(i == WARMUP_MMS - 1), skip_group_check=True,
            )

        # --- input loads: few large multi-dim-AP transfers ------------------
        # sync (HWDGE ring 1): xt in four 1MB t-blocks, first-needed first.
        xt_r = xt.rearrange("(i p) t -> p i t", p=128)
        xt_v = xt_big[:].rearrange("p (i t) -> p i t", t=T)
        for blk in range(NG):
            ts = slice(TQ * blk, TQ * (blk + 1))
            nc.sync.dma_start(xt_v[:, :, ts], xt_r[:, :, ts])
        # scalar (HWDGE ring 2, after its act-table load): weights, in
        # priority order -- queue FIFO keeps wv/wp from competing with the
        # critical wqk for HBM bandwidth.
        nc.scalar.dma_start(
            wqk_big[:].rearrange("p (i c) -> p i c", c=512),
            wqkt.rearrange("(i p) c -> p i c", p=128),
        )
        nc.scalar.dma_start(
            wv_big[:].rearrange("p (i c) -> p i c", c=256),
            wvt.rearrange("(i p) c -> p i c", p=128),
        )
        nc.scalar.dma_start(
            wp_big[:].rearrange("p (i c) -> p i c", c=C),
            wpt.rearrange("(i p) c -> p i c", p=128),
        )
        # gpsimd (SWDGE): just the tiny bias pack (needed by the first qk
        # group's bias add, lands ~2us).
        nc.gpsimd.dma_start(bias_t[:], bias[:])

        # Single causal strip mask [128, 128]: keep iff local tq >= local tk.
        maskd = res.tile([128, 128], BF16_DT, tag="maskd", name="maskd")
        nc.gpsimd.memset(maskd[:], 1.0)
        nc.gpsimd.affine_select(
            out=maskd[:], in_=maskd[:], compare_op=is_ge, fill=0.0,
            base=0, pattern=[[1, 128]], channel_multiplier=-1,
        )

        # Ones row (lane 64, matching the av denominator row) for the K=1
        # broadcast matmuls.
        ones_t = res.tile([65, 64], BF16_DT, tag="ones_t", name="ones_t")
        nc.vector.memset(ones_t[:], 1.0)

        # qT/kT in [head-channel, t] layout: tile p holds heads (2p, 2p+1).
        qk_sb = [
            res.tile([128, T], BF16_DT, tag=f"qk{i}", name=f"qk{i}") for i in range(4)
        ]
        # V natural [t, d] with a ones column after each head: 4*(64+1) cols.
        # 65 cols per head (64 v + ones) + 64 zero pad cols so every AV
        # lhsT slice can be 128 columns wide (enables FWL / hidden LDWEIGHTS).
        v_sb = []
        for i in range(NKT):
            t = res.tile([128, 324], BF16_DT, tag=f"v{i}", name=f"v{i}")
            nc.gpsimd.memset(
                t[:, 0:260].rearrange("p (h c) -> p h c", c=65)[:, :, 64:65], 1.0
            )
            nc.gpsimd.memset(t[:, 260:324], 0.0)
            v_sb.append(t)
        att_sb = [
            res.tile([128, T], BF16_DT, tag=f"att{i}", name=f"att{i}")
            for i in range(2)
        ]

        out_r = out_t.rearrange("(i p) t -> p i t", p=128)

        # --- filler work: qkv projections + output projection --------------
        emitted = set()

        def emit_qk_group(jt, g, pool=None):
            pool = pool or qv_ps
            ps = pool.tile([128, 512], FP32, tag="bp" if pool is bp_ps else "qv",
                           name=f"qkps{jt}_{g}")
            for ct in range(NC_):
                nc.tensor.matmul(
                    ps[:],
                    lhsT=wqk_t[ct][:, 128 * jt : 128 * (jt + 1)],
                    rhs=xt_t[ct][:, TQ * g : TQ * (g + 1)],
                    start=(ct == 0),
                    stop=(ct == NC_ - 1),
                )

            nc.vector.tensor_scalar(
                qk_sb[jt][:, TQ * g : TQ * (g + 1)], ps[:], bqk_t[jt][:], None,
                op0=add,
            )

        def emit_v_group(tt, pool=None):
            pool = pool or qv_ps
            ps = pool.tile([128, 512], FP32, tag="bp" if pool is bp_ps else "qv",
                           name=f"vps{tt}")
            for ct in range(NC_):
                nc.tensor.matmul(
                    ps[:, 0:256],
                    lhsT=xt_t[ct][:, 128 * tt : 128 * (tt + 1)],
                    rhs=wv_t[ct][:],
                    start=(ct == 0),
                    stop=(ct == NC_ - 1),
                )

            vt = v_sb[tt]
            nc.vector.tensor_tensor(
                out=vt[:, 0:260].rearrange("p (h c) -> p h c", c=65)[:, :, 0:64],
                in0=ps[:, 0:256].rearrange("p (h c) -> p h c", c=64),
                in1=bv_t.rearrange("p (h c) -> p h c", c=64),
                op=add,
            )

        def emit_proj_group(jt, g, pool=None, tag="bp", scalar_bias=False):
            tqs = slice(TQ * g, TQ * (g + 1))
            pp = (pool or bp_ps).tile([128, 512], FP32, tag=tag, name=f"pj{g}{jt}")
            nc.tensor.matmul(
                pp[:], lhsT=wp_t[0][:, 128 * jt : 128 * (jt + 1)],
                rhs=att_sb[0][:, tqs], start=True, stop=False,
            )
            nc.tensor.matmul(
                pp[:], lhsT=wp_t[1][:, 128 * jt : 128 * (jt + 1)],
                rhs=att_sb[1][:, tqs], start=False, stop=True,
            )
            if scalar_bias:
                nc.scalar.activation(
                    osb_t[g][:, TQ * jt : TQ * (jt + 1)], pp[:],
                    mybir.ActivationFunctionType.Identity, bias=bp_t[jt][:],
                )
            else:
                nc.vector.tensor_scalar(
                    osb_t[g][:, TQ * jt : TQ * (jt + 1)], pp[:], bp_t[jt][:],
                    None, op0=add,
                )
            # Stream the finished tq-group back: one DMA per g (g=3 quartered
            # across both HWDGE rings so the tail transfers overlap the
            # remaining proj matmuls).
            osb_v = osb_t[g][:].rearrange("p (i t) -> p i t", t=TQ)
            if g < 3:
                if jt == 7:
                    nc.sync.dma_start(out_r[:, :, tqs], osb_v)
            elif jt % 2 == 1:
                q = slice(jt - 1, jt + 1)
                eng = nc.sync if jt % 4 == 1 else nc.scalar
                eng.dma_start(out_r[:, q, tqs], osb_v[:, q, :])

        work_q = deque()

        # Dummy zero-matmuls to keep the PE clock gate open when real filler
        # runs dry (late units).
        hb_n = [0]

        def heartbeat(n=2, pool=None, tag="qv"):
            t = (pool or qv_ps).tile([128, 512], FP32, tag=tag,
                                     name=f"hb{hb_n[0]}")
            hb_n[0] += 1
            for i in range(n):
                nc.tensor.matmul(
                    t[:], lhsT=warm_sb[:, 0:128], rhs=warm_sb[:],
                    start=(i == 0), stop=(i == n - 1), skip_group_check=True,
                )

        def emit_item(item, pool=None):
            if item[0] == "qk":
                emit_qk_group(item[1], item[2], pool=pool)
            elif item[0] == "v":
                emit_v_group(item[1], pool=pool)
            else:
                emit_proj_group(item[1], item[2])
            emitted.add(item)

        def pop_one(force=False):
            if work_q:
                emit_item(work_q.popleft())

        def drain_until(needed):
            for item in needed:
                while item not in emitted:
                    emit_item(work_q.popleft())

        # prologue: enough qkv for unit (0, 0), rest queued in dep-safe order.
        # Alternate PSUM banks (qv/bp) so the DVE bias-add of one group
        # overlaps the matmuls of the next.
        for n, item in enumerate([("qk", 0, 0), ("qk", 2, 0), ("v", 0),
                                  ("v", 1), ("v", 2), ("v", 3)]):
            emit_item(item, pool=(bp_ps if n % 2 else qv_ps))
        work_q.extend([("qk", 1, 0), ("qk", 3, 0)])
        for gg in range(1, NG):
            work_q.extend(
                [("qk", 2, gg), ("qk", 0, gg), ("qk", 3, gg), ("qk", 1, gg)]
                + [("v", 4 * gg + i) for i in range(4)]
            )

        # --- attention: software-pipelined units -----------------------------
        def norm_pre(g, p, av_full):
            """Evacuate the packed den row [den_e | den_o] to bf16 SBUF."""
            den_b = riv_pool.tile([65, 1024], BF16_DT, tag="lr", name=f"dn{g}{p}")
            nc.vector.tensor_copy(out=den_b[64:65, :], in_=av_full[64:65, :])
            return den_b

        def norm_post(g, p, av_full, den_b):
            """Broadcast den across 64 partitions (K=1 matmul), invert with
            the fast DVE reciprocal, normalize; enqueues proj for p==1.
            Odd head first so its cross-partition SBUF DMA overlaps the even
            head's normalize chain."""
            tqs = slice(TQ * g, TQ * (g + 1))
            last = (g == 3 and p == 1)
            bc_o = bp_ps.tile([64, 512], FP32, tag="bp", name=f"bco{g}{p}")
            nc.tensor.matmul(
                bc_o[:], lhsT=ones_t[64:65, :], rhs=den_b[64:65, 512:1024],
                start=True, stop=True, tile_position=(64, 0),
            )
            bcs_o = bcs_pool.tile([64, 512], FP32, tag="bcs", name=f"bcso{g}{p}")
            nc.vector.reciprocal_approx_fast(out=bcs_o[:], in_=bc_o[:])
            scr = scr_pool.tile([64, 512], BF16_DT, tag="scr", name=f"scr{g}{p}")
            nc.vector.tensor_tensor(
                out=scr[:], in0=av_full[0:64, 512:1024], in1=bcs_o[:], op=mult
            )
            (nc.scalar if last else nc.sync).dma_start(
                att_sb[p][64:128, tqs], scr[:]
            )
            bc_e = bp_ps.tile([64, 512], FP32, tag="bp", name=f"bce{g}{p}")
            nc.tensor.matmul(
                bc_e[:], lhsT=ones_t[64:65, :], rhs=den_b[64:65, 0:512],
                start=True, stop=True, tile_position=(64, 0),
            )
            bcs_e = bcs_pool.tile([64, 512], FP32, tag="bcs", name=f"bcse{g}{p}")
            nc.vector.reciprocal_approx_fast(out=bcs_e[:], in_=bc_e[:])
            nc.vector.tensor_tensor(
                out=att_sb[p][0:64, tqs], in0=av_full[0:64, 0:512], in1=bcs_e[:],
                op=mult,
            )
            if p == 1:
                work_q.extend([("proj", jt, g) for jt in range(8)])

        pending_norm = None
        carry = []
        for g, p in [(0, 0), (0, 1), (1, 0), (1, 1), (2, 0), (2, 1),
                     (3, 0), (3, 1)]:
                nkt = 4 * (g + 1)
                h_e, h_o = 2 * p, 2 * p + 1
                q_t, k_t = qk_sb[p], qk_sb[2 + p]
                tq0 = TQ * g
                drain_until(
                    [("qk", p, g)]
                    + [("qk", 2 + p, gg) for gg in range(g + 1)]
                    + [("v", t) for t in range(nkt)]
                )
                s_tiles = {}
                p_tiles = {}
                av_full = None

                def lo_of(kt, g=g):
                    i = kt - 4 * g
                    return 128 * i if i > 0 else 0

                def scores(kt, g=g, q_t=q_t, k_t=k_t, tq0=tq0, p=p):
                    lo = lo_of(kt, g)
                    s_pair = sc_ps.tile([128, 1024], FP32, tag="sc",
                                        name=f"s{g}{p}{kt}")
                    kts = slice(128 * kt, 128 * (kt + 1))
                    rq = slice(tq0 + lo, tq0 + 512)
                    nc.tensor.matmul(
                        s_pair[:, lo:512], lhsT=k_t[0:64, kts], rhs=q_t[0:64, rq],
                        start=True, stop=True,
                    )
                    nc.tensor.matmul(
                        s_pair[:, 512 + lo : 1024], lhsT=k_t[64:128, kts],
                        rhs=q_t[64:128, rq], start=True, stop=True,
                        tile_position=(64, 0),
                    )
                    s_tiles[kt] = s_pair

                def expmask(kt, g=g, p=p):
                    lo = lo_of(kt, g)
                    s_pair = s_tiles.pop(kt)
                    p_pair = pt_pool.tile([128, 1024], BF16_DT, tag="pt",
                                          name=f"p{g}{p}{kt}")
                    s3 = s_pair[:].rearrange("p (h c) -> p h c", c=512)[:, :, lo:512]
                    p3 = p_pair[:].rearrange("p (h c) -> p h c", c=512)[:, :, lo:512]
                    nc.scalar.activation(p3, s3, Exp, scale=0.125)
                    if kt >= 4 * g:  # diagonal: mask the leading 128-wide strip
                        pm = p_pair[:].rearrange("p (h c) -> p h c", c=512)[
                            :, :, lo : lo + 128
                        ]
                        mk = maskd[:, None, 0:128].to_broadcast([128, 2, 128])
                        nc.gpsimd.tensor_tensor(out=pm, in0=pm, in1=mk, op=mult)
                    p_tiles[kt] = p_pair

                def av_mm(kt, av, pt, g=g, nkt=nkt, h_e=h_e, h_o=h_o):
                    lo = lo_of(kt, g)
                    p_pair = pt.pop(kt)
                    nc.tensor.matmul(
                        av[0:128, lo:512],
                        lhsT=v_sb[kt][:, 65 * h_e : 65 * h_e + 128],
                        rhs=p_pair[:, lo:512], start=(kt == 0),
                        stop=(kt == nkt - 1), skip_group_check=True,
                    )
                    nc.tensor.matmul(
                        av[0:128, 512 + lo : 1024],
                        lhsT=v_sb[kt][:, 65 * h_o : 65 * h_o + 128],
                        rhs=p_pair[:, 512 + lo : 1024], start=(kt == 0),
                        stop=(kt == nkt - 1), skip_group_check=True,
                    )

                def drain_carry(n=1):
                    for _ in range(n):
                        if carry:
                            carry.pop(0)()

                # Last `depth` AV matmuls of the previous unit interleave
                # with this unit's score/exp prologue, so the previous exp
                # chain finishes while the PE stays on fresh scores.
                depth = min(DEPTH, nkt)
                for kt in range(2):
                    scores(kt)
                    drain_carry()
                for kt in range(2):
                    expmask(kt)
                    drain_carry()
                for kt2 in range(2, depth, 2):
                    scores(kt2)
                    scores(kt2 + 1)
                    expmask(kt2)
                    expmask(kt2 + 1)
                    drain_carry(2)
                drain_carry(len(carry))
                if pending_norm is not None:
                    den_b = norm_pre(*pending_norm)
                    for _ in range(BOUNDARY_POPS):
                        pop_one()
                    norm_post(*pending_norm, den_b)
                av_full = av_ps.tile([128, 1024], FP32, tag="av",
                                     name=f"av{g}{p}")
                # kt handled in pairs: the two score-pair matmuls issue
                # back-to-back, then both AV pairs -- same-shape matmuls stay
                # adjacent so their LDWEIGHTS hide in the background buffer.
                for kt2 in range(depth, nkt, 2):
                    scores(kt2)
                    scores(kt2 + 1)
                    expmask(kt2)
                    expmask(kt2 + 1)
                    if work_q:
                        pop_one()
                    elif g >= 2:
                        heartbeat(4 if g == 2 else 6)
                    av_mm(kt2 - depth, av_full, p_tiles)
                    av_mm(kt2 - depth + 1, av_full, p_tiles)
                carry = [
                    (lambda kt=kt, av=av_full, pt=p_tiles, f=av_mm:
                     f(kt, av, pt))
                    for kt in range(nkt - depth, nkt)
                ]
                pending_norm = (g, p, av_full)

        # epilogue: drain the last unit's AV carry with heartbeats covering
        # the exp chain, then normalize and run the g=3 proj groups (these
        # rotate through the now-idle 2-bank score slots for more overlap)
        while carry:
            carry.pop(0)()
            heartbeat(4)
        den_b = norm_pre(*pending_norm)
        norm_post(*pending_norm, den_b)
        ep_pools = [(sc_ps, "sc"), (bp_ps, "bp"), (av_ps, "av")]
        np_ = 0
        while work_q:
            item = work_q.popleft()
            if item[0] == "proj":
                pool, tag = ep_pools[np_ % 3]
                emit_proj_group(item[1], item[2], pool=pool, tag=tag,
                                scalar_bias=(np_ % 2 == 1))
                np_ += 1
                emitted.add(item)
            else:
                emit_item(item)

    nc.compile()
    return nc


_NC_CACHE = None


def _get_nc():
    global _NC_CACHE
    if _NC_CACHE is None:
        _NC_CACHE = build_bass()
    return _NC_CACHE


def make_in_maps(x, w_attn, b_attn, w_proj, b_proj):
    """Host-side sharding: slice/transpose/cast the full inputs per core."""
    x = np.asarray(x, dtype=np.float32)
    w_attn = np.asarray(w_attn, dtype=np.float32)
    b_attn = np.asarray(b_attn, dtype=np.float32)
    w_proj = np.asarray(w_proj, dtype=np.float32)
    b_proj = np.asarray(b_proj, dtype=np.float32)
    in_maps = []
    for core in range(N_CORES):
        b = core // 4
        heads = [4 * (core % 4) + i for i in range(HEADS_PER_CORE)]
        ch = np.concatenate([np.arange(h * DH, (h + 1) * DH) for h in heads])
        idx_qk = np.concatenate([ch, C + ch])
        idx_v = 2 * C + ch
        bias_all = np.empty((128, 268), dtype=np.float32)
        bias_all[:, 0:4] = b_attn[idx_qk].reshape(4, 128).T
        bias_all[:, 4:12] = (b_proj / 4.0).reshape(8, 128).T
        bias_all[:, 12:268] = np.tile(b_attn[idx_v][None, :], (128, 1))
        in_maps.append(
            {
                "xt": np.ascontiguousarray(x[b].T).astype(BF16),
                "wqkt": np.ascontiguousarray(w_attn[idx_qk].T).astype(BF16),
                "wvt": np.ascontiguousarray(w_attn[idx_v].T).astype(BF16),
                "wpt": np.ascontiguousarray(w_proj[:, ch].T).astype(BF16),
                "bias": bias_all,
            }
        )
    return in_maps


def assemble_output(results):
    out = np.zeros((B, T, C), dtype=np.float32)
    for core in range(N_CORES):
        out[core // 4] += np.asarray(results[core]["out_t"], dtype=np.float32).T
    return out


def run(inputs, trace=False, trace_cores=None, tmpdir=None):
    """Run on hardware; returns (output, BassKernelResults)."""
    _ensure_axon_hooks_stub()
    from concourse.bass_utils import run_bass_kernel_spmd

    nc = _get_nc()
    in_maps = make_in_maps(**inputs)
    kw = {}
    if trace:
        kw.update(trace=True, trace_cores=trace_cores, tmpdir=tmpdir)
    res = run_bass_kernel_spmd(nc, in_maps, core_ids=list(range(N_CORES)), **kw)
    return assemble_output(res.results), res


def kernel(x, w_attn, b_attn, w_proj, b_proj):
    out, _ = run(
        dict(x=x, w_attn=w_attn, b_attn=b_attn, w_proj=w_proj, b_proj=b_proj)
    )
    return out


# revision 18
# speedup vs baseline: 1.0502x; 1.0502x over previous
"""Causal self-attention (B=2, T=2048, C=1024, H=16) on 8 TRN2 NeuronCores.

Sharding: core c -> batch b = c // 4, head group = heads [4*(c%4) .. 4*(c%4)+4).
Each core computes qkv for its 4 heads on its batch, causal attention, and a
row-parallel partial of the output projection (over its 256 head channels).
The host sums the 4 partials per batch; b_proj/4 is added on-device so the sum
reproduces a single b_proj add.  Output partials stream back in bf16.

All device tensors are pre-transposed on the host so the kernel never
transposes on-chip:
  xt   [C, T]    = x[b].T                     (bf16)
  wqkt [C, 512]  = w_attn[qk rows].T          (bf16)  cols: q_h0 q_h1 q_h2 q_h3 k_h0..k_h3
  wvt  [C, 256]  = w_attn[v rows].T           (bf16)
  wpt  [256, C]  = w_proj[:, head cols].T     (bf16)
  bias [128,268] = packed bqk(4) | bp(8) | bv(256) columns (fp32)
  out_t[C, T]    = partial (x @ w_proj.T).T   (bf16)

Round 3 (vs the round-2 baseline at ~194us):
  * input DMAs coalesced into 7 big multi-dim-AP transfers (xt in four
    1MB t-blocks on sync, wqk/wv on scalar, bias/wp on gpsimd) so the
    first qk group can start ~6us in instead of ~22us;
  * softmax 1/den moved off ScalarE: DVE copies the PSUM den row to bf16,
    a K=1 matmul broadcasts den across 64 partitions, and
    reciprocal_approx_fast (custom DVE op, ~18 bits) inverts it -- the
    Ln/Exp activation chain is gone (ScalarE now runs score exps only);
  * av pair kept in one 2-bank PSUM tile [128,1024] (den row spans both
    heads, one DVE copy per unit);
  * proj outputs accumulate per tq-group into resident [128,4096] bf16
    tiles, written back with one DMA per group (g=3 split in halves), so
    the tail is a short dense burst instead of 8 DMA-gated proj groups.

On-chip dataflow (per head pair, row/col layouts chosen so the TensorE
contraction dim is always the partition dim and no transposes are needed):
  qT,kT [d, t] -> S^T[tk, tq] (two heads packed in the 128-wide array via
  row tiling) -> exp on ScalarE (scale=1/8 folded in) -> causal mask via
  static 0/1 mask multiply on DVE -> AV matmul with V augmented by a ones
  column (denominator accumulates in row 64 of PSUM for free) -> bcast +
  reciprocal -> normalize -> projection (stays transposed).
"""

import os
import sys
import types

import numpy as np
import ml_dtypes

import concourse.bass as bass
import concourse.mybir as mybir
import concourse.tile as tile
from concourse import bacc
from concourse.hw_specs import get_activation_tables

BF16 = ml_dtypes.bfloat16


class _Bacc(bacc.Bacc):
    """Bacc that steers Exp/Ln activations to the combined
    natural_log_exp_and_others table set so the kernel never swaps
    activation tables (set ids keep their act_info.json positions)."""

    def insert_act_table_loads(self):
        import bass_rust as _br
        import concourse.mybir as _mybir

        has_activation = any(
            isinstance(i, _mybir.InstActivation)
            for b in self.main_func.blocks
            for i in b.instructions
        )
        if not has_activation:
            return
        combined = {"natural_log_exp_and_others"}
        steer = {_mybir.ActivationFunctionType.Exp, _mybir.ActivationFunctionType.Ln}
        tables = []
        for name, fns in get_activation_tables(self.m.arch).items():
            if name not in combined:
                fns = {f for f in fns if f not in steer}
            tables.append((name, set(fns)))
        _br.insert_act_table_loads(self, tables)

B, T, C = 2, 2048, 1024
H = 16
DH = 64
N_CORES = 8
HEADS_PER_CORE = 4
TQ = 512          # tq tile (moving dim of scores/AV matmuls)
TK = 128          # tk tile (PSUM partition dim of S^T)
NG = T // TQ      # 4 tq tiles
NKT = T // TK     # 16 tk tiles
NC_ = C // 128    # 8 contraction tiles for the qkv matmuls
FP32 = mybir.dt.float32
BF16_DT = mybir.dt.bfloat16
import os as _os
DEPTH = int(_os.environ.get("K_DEPTH", "4"))
POPS_EVERY = int(_os.environ.get("K_POPS_EVERY", "2"))
BOUNDARY_POPS = int(_os.environ.get("K_BPOPS", "2"))
WARMUP_MMS = int(_os.environ.get("K_WARMUP", "40"))
PT_BUFS = int(_os.environ.get("K_PT_BUFS", "8"))


def _ensure_axon_hooks_stub():
    """bass_utils imports antenv.axon_hooks when trace is requested (even via
    the BASS_TRACE env var). The container's antenv stub lacks that module, so
    install a minimal one to keep the no-trace fallback path working."""
    try:
        import antenv  # noqa: F401
    except ImportError:
        return
    if "antenv.axon_hooks" in sys.modules:
        return
    try:
        import antenv.axon_hooks  # noqa: F401
        return
    except ImportError:
        pass
    mod = types.ModuleType("antenv.axon_hooks")
    mod._hook = None

    def set_axon_ntff_profile_hook(h):
        mod._hook = h

    def get_axon_ntff_profile_hook():
        return mod._hook

    mod.set_axon_ntff_profile_hook = set_axon_ntff_profile_hook
    mod.get_axon_ntff_profile_hook = get_axon_ntff_profile_hook
    sys.modules["antenv.axon_hooks"] = mod
    import antenv as _a

    _a.axon_hooks = mod


def build_bass():
    """Emit the single-core SPMD Bass module (same program on all 8 cores)."""
    from collections import deque
    from contextlib import ExitStack

    nc = _Bacc("TRN2", target_bir_lowering=False, debug=False)

    xt = nc.declare_dram_parameter("xt", [C, T], BF16_DT, isOutput=False).ap()
    wqkt = nc.declare_dram_parameter("wqkt", [C, 512], BF16_DT, isOutput=False).ap()
    wvt = nc.declare_dram_parameter("wvt", [C, 256], BF16_DT, isOutput=False).ap()
    wpt = nc.declare_dram_parameter("wpt", [256, C], BF16_DT, isOutput=False).ap()
    bias = nc.declare_dram_parameter("bias", [128, 268], FP32, isOutput=False).ap()
    out_t = nc.declare_dram_parameter("out_t", [C, T], BF16_DT, isOutput=True).ap()

    Exp = mybir.ActivationFunctionType.Exp
    mult = mybir.AluOpType.mult
    add = mybir.AluOpType.add
    is_ge = mybir.AluOpType.is_ge

    with tile.TileContext(nc) as tc, ExitStack() as ctx:
        res = ctx.enter_context(tc.tile_pool(name="resident", bufs=1))

        # --- resident tiles (one big tile per input, sliced into views) ----
        xt_big = res.tile([128, NC_ * T], BF16_DT, tag="xt", name="xt_big")
        xt_t = [xt_big[:, T * i : T * (i + 1)] for i in range(NC_)]
        wqk_big = res.tile([128, NC_ * 512], BF16_DT, tag="wqk", name="wqk_big")
        wqk_t = [wqk_big[:, 512 * i : 512 * (i + 1)] for i in range(NC_)]
        wv_big = res.tile([128, NC_ * 256], BF16_DT, tag="wv", name="wv_big")
        wv_t = [wv_big[:, 256 * i : 256 * (i + 1)] for i in range(NC_)]
        wp_big = res.tile([128, 2 * C], BF16_DT, tag="wp", name="wp_big")
        wp_t = [wp_big[:, C * i : C * (i + 1)] for i in range(2)]
        bias_t = res.tile([128, 268], FP32, tag="bias", name="bias_t")
        bqk_t = [bias_t[:, j : j + 1] for j in range(4)]
        bp_t = [bias_t[:, 4 + j : 5 + j] for j in range(8)]
        bv_t = bias_t[:, 12:268]
        osb_t = [res.tile([128, NC_ * TQ], BF16_DT, tag=f"osb{g}", name=f"osb{g}")
                 for g in range(NG)]

        # --- PE warm-up first: dense zero matmuls while the DMAs stream in,
        # so the HAM clock gate opens before real compute starts ------------
        warm_sb = res.tile([128, 512], BF16_DT, tag="warm", name="warm_sb")
        nc.vector.memset(warm_sb[:], 0.0)

        sc_ps = ctx.enter_context(tc.tile_pool(name="sc_ps", bufs=2, space="PSUM"))
        av_ps = ctx.enter_context(tc.tile_pool(name="av_ps", bufs=1, space="PSUM"))
        qv_ps = ctx.enter_context(tc.tile_pool(name="qv_ps", bufs=1, space="PSUM"))
        bp_ps = ctx.enter_context(tc.tile_pool(name="bp_ps", bufs=1, space="PSUM"))
        pt_pool = ctx.enter_context(tc.tile_pool(name="pt_pool", bufs=PT_BUFS))
        riv_pool = ctx.enter_context(tc.tile_pool(name="riv", bufs=2))
        bcs_pool = ctx.enter_context(tc.tile_pool(name="bcs", bufs=2))
        scr_pool = ctx.enter_context(tc.tile_pool(name="scr", bufs=2))

        warm_ps = qv_ps.tile([128, 512], FP32, tag="qv", name="warm_ps")
        for i in range(WARMUP_MMS):
            nc.tensor.matmul(
                warm_ps[:], lhsT=warm_sb[:, 0:128], rhs=warm_sb[:],
                start=(i == 0), stop=(i == WARMUP_MMS - 1), skip_group_check=True,
            )

        # --- input loads: few large multi-dim-AP transfers ------------------
        # sync (HWDGE ring 1): xt in four 1MB t-blocks, first-needed first.
        xt_r = xt.rearrange("(i p) t -> p i t", p=128)
        xt_v = xt_big[:].rearrange("p (i t) -> p i t", t=T)
        for blk in range(NG):
            ts = slice(TQ * blk, TQ * (blk + 1))
            nc.sync.dma_start(xt_v[:, :, ts], xt_r[:, :, ts])
        # scalar (HWDGE ring 2, after its act-table load): weights, in
        # priority order -- queue FIFO keeps wv/wp from competing with the
        # critical wqk for HBM bandwidth.
        nc.scalar.dma_start(
            wqk_big[:].rearrange("p (i c) -> p i c", c=512),
            wqkt.rearrange("(i p) c -> p i c", p=128),
        )
        nc.scalar.dma_start(
            wv_big[:].rearrange("p (i c) -> p i c", c=256),
            wvt.rearrange("(i p) c -> p i c", p=128),
        )
        nc.scalar.dma_start(
            wp_big[:].rearrange("p (i c) -> p i c", c=C),
            wpt.rearrange("(i p) c -> p i c", p=128),
        )
        # gpsimd (SWDGE): just the tiny bias pack (needed by the first qk
        # group's bias add, lands ~2us).
        nc.gpsimd.dma_start(bias_t[:], bias[:])

        # Single causal strip mask [128, 128]: keep iff local tq >= local tk.
        maskd = res.tile([128, 128], BF16_DT, tag="maskd", name="maskd")
        nc.gpsimd.memset(maskd[:], 1.0)
        nc.gpsimd.affine_select(
            out=maskd[:], in_=maskd[:], compare_op=is_ge, fill=0.0,
            base=0, pattern=[[1, 128]], channel_multiplier=-1,
        )

        # Ones row (lane 64, matching the av denominator row) for the K=1
        # broadcast matmuls.
        ones_t = res.tile([65, 64], BF16_DT, tag="ones_t", name="ones_t")
        nc.vector.memset(ones_t[:], 1.0)

        # qT/kT in [head-channel, t] layout: tile p holds heads (2p, 2p+1).
        qk_sb = [
            res.tile([128, T], BF16_DT, tag=f"qk{i}", name=f"qk{i}") for i in range(4)
        ]
        # V natural [t, d] with a ones column after each head: 4*(64+1) cols.
        # 65 cols per head (64 v + ones) + 64 zero pad cols so every AV
        # lhsT slice can be 128 columns wide (enables FWL / hidden LDWEIGHTS).
        v_sb = []
        for i in range(NKT):
            t = res.tile([128, 324], BF16_DT, tag=f"v{i}", name=f"v{i}")
            nc.gpsimd.memset(
                t[:, 0:260].rearrange("p (h c) -> p h c", c=65)[:, :, 64:65], 1.0
            )
            nc.gpsimd.memset(t[:, 260:324], 0.0)
            v_sb.append(t)
        att_sb = [
            res.tile([128, T], BF16_DT, tag=f"att{i}", name=f"att{i}")
            for i in range(2)
        ]

        out_r = out_t.rearrange("(i p) t -> p i t", p=128)

        # --- filler work: qkv projections + output projection --------------
        emitted = set()

        def emit_qk_group(jt, g, pool=None):
            pool = pool or qv_ps
            ps = pool.tile([128, 512], FP32, tag="bp" if pool is bp_ps else "qv",
                           name=f"qkps{jt}_{g}")
            for ct in range(NC_):
                nc.tensor.matmul(
                    ps[:],
                    lhsT=wqk_t[ct][:, 128 * jt : 128 * (jt + 1)],
                    rhs=xt_t[ct][:, TQ * g : TQ * (g + 1)],
                    start=(ct == 0),
                    stop=(ct == NC_ - 1),
                )

            nc.vector.tensor_scalar(
                qk_sb[jt][:, TQ * g : TQ * (g + 1)], ps[:], bqk_t[jt][:], None,
                op0=add,
            )

        def emit_v_group(tt, pool=None):
            pool = pool or qv_ps
            ps = pool.tile([128, 512], FP32, tag="bp" if pool is bp_ps else "qv",
                           name=f"vps{tt}")
            for ct in range(NC_):
                nc.tensor.matmul(
                    ps[:, 0:256],
                    lhsT=xt_t[ct][:, 128 * tt : 128 * (tt + 1)],
                    rhs=wv_t[ct][:],
                    start=(ct == 0),
                    stop=(ct == NC_ - 1),
                )

            vt = v_sb[tt]
            nc.vector.tensor_tensor(
                out=vt[:, 0:260].rearrange("p (h c) -> p h c", c=65)[:, :, 0:64],
                in0=ps[:, 0:256].rearrange("p (h c) -> p h c", c=64),
                in1=bv_t.rearrange("p (h c) -> p h c", c=64),
                op=add,
            )

        def emit_proj_group(jt, g, pool=None, tag="bp", scalar_bias=False):
            tqs = slice(TQ * g, TQ * (g + 1))
            pp = (pool or bp_ps).tile([128, 512], FP32, tag=tag, name=f"pj{g}{jt}")
            nc.tensor.matmul(
                pp[:], lhsT=wp_t[0][:, 128 * jt : 128 * (jt + 1)],
                rhs=att_sb[0][:, tqs], start=True, stop=False,
            )
            nc.tensor.matmul(
                pp[:], lhsT=wp_t[1][:, 128 * jt : 128 * (jt + 1)],
                rhs=att_sb[1][:, tqs], start=False, stop=True,
            )
            if scalar_bias:
                nc.scalar.activation(
                    osb_t[g][:, TQ * jt : TQ * (jt + 1)], pp[:],
                    mybir.ActivationFunctionType.Identity, bias=bp_t[jt][:],
                )
            else:
                nc.vector.tensor_scalar(
                    osb_t[g][:, TQ * jt : TQ * (jt + 1)], pp[:], bp_t[jt][:],
                    None, op0=add,
                )
            # Stream the finished tq-group back: one DMA per g (g=3 quartered
            # across both HWDGE rings so the tail transfers overlap the
            # remaining proj matmuls).
            osb_v = osb_t[g][:].rearrange("p (i t) -> p i t", t=TQ)
            if g < 3:
                if jt == 7:
                    nc.sync.dma_start(out_r[:, :, tqs], osb_v)
            elif jt % 2 == 1:
                q = slice(jt - 1, jt + 1)
                eng = nc.sync if jt % 4 == 1 else nc.scalar
                eng.dma_start(out_r[:, q, tqs], osb_v[:, q, :])

        work_q = deque()
        proj_q = deque()
        proj_n = [0]

        # Dummy zero-matmuls to keep the PE clock gate open when real filler
        # runs dry (late units).
        hb_n = [0]

        def heartbeat(n=2, pool=None, tag="qv"):
            t = (pool or qv_ps).tile([128, 512], FP32, tag=tag,
                                     name=f"hb{hb_n[0]}")
            hb_n[0] += 1
            for i in range(n):
                nc.tensor.matmul(
                    t[:], lhsT=warm_sb[:, 0:128], rhs=warm_sb[:],
                    start=(i == 0), stop=(i == n - 1), skip_group_check=True,
                )

        def emit_item(item, pool=None):
            if item[0] == "qk":
                emit_qk_group(item[1], item[2], pool=pool)
            elif item[0] == "v":
                emit_v_group(item[1], pool=pool)
            else:
                emit_proj_group(item[1], item[2])
            emitted.add(item)

        def pop_one(force=False):
            if work_q:
                emit_item(work_q.popleft())
                return True
            if proj_q:
                jt, g = proj_q.popleft()
                emit_proj_group(jt, g, pool=(bp_ps if proj_n[0] % 2 else qv_ps),
                                tag=("bp" if proj_n[0] % 2 else "qv"))
                proj_n[0] += 1
                emitted.add(("proj", jt, g))
                return True
            return False

        def drain_until(needed):
            for item in needed:
                while item not in emitted:
                    emit_item(work_q.popleft())

        # prologue: enough qkv for unit (0, 0), rest queued in dep-safe order.
        # Alternate PSUM banks (qv/bp) so the DVE bias-add of one group
        # overlaps the matmuls of the next.
        for n, item in enumerate([("qk", 0, 0), ("qk", 2, 0), ("v", 0),
                                  ("v", 1), ("v", 2), ("v", 3)]):
            emit_item(item, pool=(bp_ps if n % 2 else qv_ps))
        work_q.extend([("qk", 1, 0), ("qk", 3, 0)])
        for gg in range(1, NG):
            work_q.extend(
                [("qk", 2, gg), ("qk", 0, gg), ("qk", 3, gg), ("qk", 1, gg)]
                + [("v", 4 * gg + i) for i in range(4)]
            )

        # --- attention: software-pipelined units -----------------------------
        def norm_pre(g, p, av_full):
            """Evacuate the packed den row [den_e | den_o] to bf16 SBUF."""
            den_b = riv_pool.tile([65, 1024], BF16_DT, tag="lr", name=f"dn{g}{p}")
            nc.vector.tensor_copy(out=den_b[64:65, :], in_=av_full[64:65, :])
            return den_b

        def norm_post(g, p, av_full, den_b):
            """Broadcast den across 64 partitions (K=1 matmul), invert with
            the fast DVE reciprocal, normalize; enqueues proj for p==1.
            Odd head first so its cross-partition SBUF DMA overlaps the even
            head's normalize chain."""
            tqs = slice(TQ * g, TQ * (g + 1))
            last = (g == 3 and p == 1)
            bc_o = bp_ps.tile([64, 512], FP32, tag="bp", name=f"bco{g}{p}")
            nc.tensor.matmul(
                bc_o[:], lhsT=ones_t[64:65, :], rhs=den_b[64:65, 512:1024],
                start=True, stop=True, tile_position=(64, 0),
            )
            bcs_o = bcs_pool.tile([64, 512], FP32, tag="bcs", name=f"bcso{g}{p}")
            nc.vector.reciprocal_approx_fast(out=bcs_o[:], in_=bc_o[:])
            scr = scr_pool.tile([64, 512], BF16_DT, tag="scr", name=f"scr{g}{p}")
            nc.vector.tensor_tensor(
                out=scr[:], in0=av_full[0:64, 512:1024], in1=bcs_o[:], op=mult
            )
            (nc.scalar if last else nc.sync).dma_start(
                att_sb[p][64:128, tqs], scr[:]
            )
            bc_e = bp_ps.tile([64, 512], FP32, tag="bp", name=f"bce{g}{p}")
            nc.tensor.matmul(
                bc_e[:], lhsT=ones_t[64:65, :], rhs=den_b[64:65, 0:512],
                start=True, stop=True, tile_position=(64, 0),
            )
            bcs_e = bcs_pool.tile([64, 512], FP32, tag="bcs", name=f"bcse{g}{p}")
            nc.vector.reciprocal_approx_fast(out=bcs_e[:], in_=bc_e[:])
            nc.vector.tensor_tensor(
                out=att_sb[p][0:64, tqs], in0=av_full[0:64, 0:512], in1=bcs_e[:],
                op=mult,
            )
            if p == 1:
                proj_q.extend([(jt, g) for jt in range(8)])

        pending_norm = None
        carry = []
        for g, p in [(0, 0), (0, 1), (1, 0), (1, 1), (2, 0), (2, 1),
                     (3, 0), (3, 1)]:
                nkt = 4 * (g + 1)
                h_e, h_o = 2 * p, 2 * p + 1
                q_t, k_t = qk_sb[p], qk_sb[2 + p]
                tq0 = TQ * g
                drain_until(
                    [("qk", p, g)]
                    + [("qk", 2 + p, gg) for gg in range(g + 1)]
                    + [("v", t) for t in range(nkt)]
                )
                s_tiles = {}
                p_tiles = {}
                av_full = None

                def lo_of(kt, g=g):
                    i = kt - 4 * g
                    return 128 * i if i > 0 else 0

                def scores(kt, g=g, q_t=q_t, k_t=k_t, tq0=tq0, p=p):
                    lo = lo_of(kt, g)
                    s_pair = sc_ps.tile([128, 1024], FP32, tag="sc",
                                        name=f"s{g}{p}{kt}")
                    kts = slice(128 * kt, 128 * (kt + 1))
                    rq = slice(tq0 + lo, tq0 + 512)
                    nc.tensor.matmul(
                        s_pair[:, lo:512], lhsT=k_t[0:64, kts], rhs=q_t[0:64, rq],
                        start=True, stop=True,
                    )
                    nc.tensor.matmul(
                        s_pair[:, 512 + lo : 1024], lhsT=k_t[64:128, kts],
                        rhs=q_t[64:128, rq], start=True, stop=True,
                        tile_position=(64, 0),
                    )
                    s_tiles[kt] = s_pair

                def expmask(kt, g=g, p=p):
                    lo = lo_of(kt, g)
                    s_pair = s_tiles.pop(kt)
                    p_pair = pt_pool.tile([128, 1024], BF16_DT, tag="pt",
                                          name=f"p{g}{p}{kt}")
                    s3 = s_pair[:].rearrange("p (h c) -> p h c", c=512)[:, :, lo:512]
                    p3 = p_pair[:].rearrange("p (h c) -> p h c", c=512)[:, :, lo:512]
                    nc.scalar.activation(p3, s3, Exp, scale=0.125)
                    if kt >= 4 * g:  # diagonal: mask the leading 128-wide strip
                        pm = p_pair[:].rearrange("p (h c) -> p h c", c=512)[
                            :, :, lo : lo + 128
                        ]
                        mk = maskd[:, None, 0:128].to_broadcast([128, 2, 128])
                        nc.gpsimd.tensor_tensor(out=pm, in0=pm, in1=mk, op=mult)
                    p_tiles[kt] = p_pair

                def av_mm(kt, av, pt, g=g, nkt=nkt, h_e=h_e, h_o=h_o):
                    lo = lo_of(kt, g)
                    p_pair = pt.pop(kt)
                    nc.tensor.matmul(
                        av[0:128, lo:512],
                        lhsT=v_sb[kt][:, 65 * h_e : 65 * h_e + 128],
                        rhs=p_pair[:, lo:512], start=(kt == 0),
                        stop=(kt == nkt - 1), skip_group_check=True,
                    )
                    nc.tensor.matmul(
                        av[0:128, 512 + lo : 1024],
                        lhsT=v_sb[kt][:, 65 * h_o : 65 * h_o + 128],
                        rhs=p_pair[:, 512 + lo : 1024], start=(kt == 0),
                        stop=(kt == nkt - 1), skip_group_check=True,
                    )

                def drain_carry(n=1):
                    for _ in range(n):
                        if carry:
                            carry.pop(0)()

                # Last `depth` AV matmuls of the previous unit interleave
                # with this unit's score/exp prologue, so the previous exp
                # chain finishes while the PE stays on fresh scores.
                depth = min(DEPTH, nkt)
                for kt in range(2):
                    scores(kt)
                    drain_carry()
                for kt in range(2):
                    expmask(kt)
                    drain_carry()
                for kt2 in range(2, depth, 2):
                    scores(kt2)
                    scores(kt2 + 1)
                    expmask(kt2)
                    expmask(kt2 + 1)
                    drain_carry(2)
                drain_carry(len(carry))
                if pending_norm is not None:
                    den_b = norm_pre(*pending_norm)
                    for _ in range(BOUNDARY_POPS):
                        pop_one()
                    norm_post(*pending_norm, den_b)
                av_full = av_ps.tile([128, 1024], FP32, tag="av",
                                     name=f"av{g}{p}")
                # kt handled in pairs: the two score-pair matmuls issue
                # back-to-back, then both AV pairs -- same-shape matmuls stay
                # adjacent so their LDWEIGHTS hide in the background buffer.
                for kt2 in range(depth, nkt, 2):
                    scores(kt2)
                    scores(kt2 + 1)
                    expmask(kt2)
                    expmask(kt2 + 1)
                    if not pop_one() and g >= 2:
                        heartbeat(4)
                    av_mm(kt2 - depth, av_full, p_tiles)
                    av_mm(kt2 - depth + 1, av_full, p_tiles)
                carry = [
                    (lambda kt=kt, av=av_full, pt=p_tiles, f=av_mm:
                     f(kt, av, pt))
                    for kt in range(nkt - depth, nkt)
                ]
                pending_norm = (g, p, av_full)

        # epilogue: drain the last unit's AV carry with heartbeats covering
        # the exp chain, then normalize and run the g=3 proj groups (these
        # rotate through the now-idle 2-bank score slots for more overlap)
        while carry:
            carry.pop(0)()
            heartbeat(2)
        den_b = norm_pre(*pending_norm)
        norm_post(*pending_norm, den_b)
        while work_q:
            emit_item(work_q.popleft())
        ep_pools = [(sc_ps, "sc"), (bp_ps, "bp"), (av_ps, "av")]
        np_ = 0
        while proj_q:
            jt, g = proj_q.popleft()
            pool, tag = ep_pools[np_ % 3]
            emit_proj_group(jt, g, pool=pool, tag=tag,
                            scalar_bias=(np_ % 2 == 1))
            np_ += 1
            emitted.add(("proj", jt, g))

    nc.compile()
    return nc


_NC_CACHE = None


def _get_nc():
    global _NC_CACHE
    if _NC_CACHE is None:
        _NC_CACHE = build_bass()
    return _NC_CACHE


def make_in_maps(x, w_attn, b_attn, w_proj, b_proj):
    """Host-side sharding: slice/transpose/cast the full inputs per core."""
    x = np.asarray(x, dtype=np.float32)
    w_attn = np.asarray(w_attn, dtype=np.float32)
    b_attn = np.asarray(b_attn, dtype=np.float32)
    w_proj = np.asarray(w_proj, dtype=np.float32)
    b_proj = np.asarray(b_proj, dtype=np.float32)
    in_maps = []
    for core in range(N_CORES):
        b = core // 4
        heads = [4 * (core % 4) + i for i in range(HEADS_PER_CORE)]
        ch = np.concatenate([np.arange(h * DH, (h + 1) * DH) for h in heads])
        idx_qk = np.concatenate([ch, C + ch])
        idx_v = 2 * C + ch
        bias_all = np.empty((128, 268), dtype=np.float32)
        bias_all[:, 0:4] = b_attn[idx_qk].reshape(4, 128).T
        bias_all[:, 4:12] = (b_proj / 4.0).reshape(8, 128).T
        bias_all[:, 12:268] = np.tile(b_attn[idx_v][None, :], (128, 1))
        in_maps.append(
            {
                "xt": np.ascontiguousarray(x[b].T).astype(BF16),
                "wqkt": np.ascontiguousarray(w_attn[idx_qk].T).astype(BF16),
                "wvt": np.ascontiguousarray(w_attn[idx_v].T).astype(BF16),
                "wpt": np.ascontiguousarray(w_proj[:, ch].T).astype(BF16),
                "bias": bias_all,
            }
        )
    return in_maps


def assemble_output(results):
    out = np.zeros((B, T, C), dtype=np.float32)
    for core in range(N_CORES):
        out[core // 4] += np.asarray(results[core]["out_t"], dtype=np.float32).T
    return out


def run(inputs, trace=False, trace_cores=None, tmpdir=None):
    """Run on hardware; returns (output, BassKernelResults)."""
    _ensure_axon_hooks_stub()
    from concourse.bass_utils import run_bass_kernel_spmd

    nc = _get_nc()
    in_maps = make_in_maps(**inputs)
    kw = {}
    if trace:
        kw.update(trace=True, trace_cores=trace_cores, tmpdir=tmpdir)
    res = run_bass_kernel_spmd(nc, in_maps, core_ids=list(range(N_CORES)), **kw)
    return assemble_output(res.results), res


def kernel(x, w_attn, b_attn, w_proj, b_proj):
    out, _ = run(
        dict(x=x, w_attn=w_attn, b_attn=b_attn, w_proj=w_proj, b_proj=b_proj)
    )
    return out
